# revision 28
# baseline (speedup 1.0000x reference)
"""Trainium2 Bass kernel for Transformer-XL style relative-position multi-head
self-attention (nn_MultiHeadedSelfAttention_35588099015524).

Sharding: batch (B=8) is data-parallel across the 8 NeuronCores; no collectives.

Math trick (v2 — compressed frequency basis): the Transformer-XL relative
shift term is
    bd[i,j] = qv_i . p_{j-i},   p_d = pe_d @ Wpos   (per head)
with pe the 512-dim sinusoid table over log-spaced frequencies w_c. The 512
basis functions {sin(w_c d), cos(w_c d)} restricted to the window
d in (-1024, 1024) are numerically rank-deficient: a least-squares fit onto
M = 128 frequencies nu_m (the top-96 w_c kept exactly + a 32-point linear
grid under them) reproduces all 512 functions to ~1e-13 (triangular-weighted
over the actual (i,j) usage counts). Writing
    F_e(d) ~= sum_m As[m,e] sin(nu_m d) + Ac[m,e] cos(nu_m d)
and applying the angle-addition formulas gives the exact-rank-256 form
    bd[i,j] = sum_m sin(nu_m j) As~[i,m] + cos(nu_m j) Ac~[i,m]
    As~[i,m] =  cos(nu_m i) Gs[i,m] + sin(nu_m i) Gc[i,m]
    Ac~[i,m] =  cos(nu_m i) Gc[i,m] - sin(nu_m i) Gs[i,m]
    Gs = qv @ (Wpos_h^T As^T),   Gc = qv @ (Wpos_h^T Ac^T)     (64 x 128 each)
so the per-score-tile contraction is 64 (qu.k) + 128 (As~) + 128 (Ac~)
= 3 matmul instructions instead of 5 (the old exact-512 sinusoid form).
Scores are computed transposed (S^T[j,i]) so softmax-normalization sums ride
along as an extra ones-column in V and no on-device transposes are needed.
The v_bias contribution rides as a per-frequency constant row added to Gs/Gc
on DVE ((v-u) @ W~ precomputed on host; G~ itself is computed from qu).
"""

import sys

sys.path.insert(0, "/opt/trn_rl_repo")

from contextlib import ExitStack  # noqa: E402

import numpy as np  # noqa: E402
import ml_dtypes  # noqa: E402

import concourse.bass as bass  # noqa: E402
from concourse import bacc, library_config  # noqa: E402
import concourse.tile as tile  # noqa: E402
from concourse import mybir  # noqa: E402
from concourse.bass_utils import run_bass_kernel_spmd  # noqa: E402

# Force every ACT function we use (Exp/Ln/Copy) to resolve to the single
# "natural_log_exp_and_others" table set — otherwise the table-load pass
# flip-flops between sets per head (~2.7us per ACT_TABLE_LOAD).
import concourse.hw_specs as _hs  # noqa: E402
import concourse.bacc as _bacc_mod  # noqa: E402

if not getattr(_hs, "_act_tables_pinned", False):
    _orig_gat = _hs.get_activation_tables

    def _pinned_gat(arch):
        tabs = _orig_gat(arch)
        keep = "natural_log_exp_and_others"
        pin = {mybir.ActivationFunctionType.Exp,
               mybir.ActivationFunctionType.Ln,
               mybir.ActivationFunctionType.Copy}
        if keep in tabs and pin <= tabs[keep]:
            for k in tabs:
                if k != keep:
                    tabs[k] = tabs[k] - pin
        return tabs

    _hs.get_activation_tables = _pinned_gat
    _bacc_mod.get_activation_tables = _pinned_gat
    _hs._act_tables_pinned = True

B, T, D = 8, 1024, 512
H, DH = 8, 64
NCORES = 8
M = 128                      # compressed frequency count (2M = 256 bd rank)
SCALE = 1.0 / np.sqrt(DH)

F32 = mybir.dt.float32
BF16 = mybir.dt.bfloat16

# knob: matmul/elementwise working dtype ("bf16" or "f32r")
MM_MODE = "bf16"


def _np_dt(mode):
    return ml_dtypes.bfloat16 if mode == "bf16" else np.float32


def _mm_dt(mode):
    return BF16 if mode == "bf16" else mybir.dt.float32r


# ---------------------------------------------------------------------------
# host-side constant precompute: frequency fit (input-independent, cached)
# ---------------------------------------------------------------------------
_FIT_CACHE = {}


def _freq_fit():
    """Least-squares fit of the 512 reference sinusoids onto M frequencies.

    Returns (nu (M,), As (M,256), Ac (M,256)) such that over the triangular-
    weighted window d in (-1024, 1024):
        sin(w_c d) ~= sum_m As[m,c] sin(nu_m d)
        cos(w_c d) ~= sum_m Ac[m,c] cos(nu_m d)
    """
    if "fit" in _FIT_CACHE:
        return _FIT_CACHE["fit"]
    c = np.arange(256)
    omega = np.exp(-np.log(10000.0) * (2.0 * c) / D)
    delta = np.arange(-(T - 1), T, dtype=np.float64)
    w = (T - np.abs(delta)) / T
    sw = np.sqrt(w)[:, None]
    c0, ngrid = M - 32, 32
    nu = np.concatenate([omega[:c0],
                         np.linspace(0, omega[c0 - 1], ngrid, endpoint=False)])
    ang_t = np.outer(delta, omega)
    Bs = np.sin(np.outer(delta, nu)) * sw
    Bc = np.cos(np.outer(delta, nu)) * sw
    As, *_ = np.linalg.lstsq(Bs, np.sin(ang_t) * sw, rcond=None)
    Ac, *_ = np.linalg.lstsq(Bc, np.cos(ang_t) * sw, rcond=None)
    _FIT_CACHE["fit"] = (nu, As, Ac)
    return _FIT_CACHE["fit"]


def build_nc(mode=MM_MODE):
    """Build the per-core Bass module (identical program on all 8 cores)."""
    DT = _mm_dt(mode)
    nc = bacc.Bacc("TRN2", target_bir_lowering=False, debug=False)

    # ---- DRAM parameters (per core) ----
    xsT_d = nc.declare_dram_parameter("xsT", [D, T], DT, isOutput=False)
    wq_d = nc.declare_dram_parameter("Wq", [D, D], DT, isOutput=False)
    wk_d = nc.declare_dram_parameter("Wk", [D, D], DT, isOutput=False)
    wv_d = nc.declare_dram_parameter("Wv", [D, D], DT, isOutput=False)
    wts_d = nc.declare_dram_parameter("WTS", [128, H * M], DT, isOutput=False)
    wtc_d = nc.declare_dram_parameter("WTC", [128, H * M], DT, isOutput=False)
    wout_d = nc.declare_dram_parameter("Wout", [D, D], DT, isOutput=False)
    ubp_d = nc.declare_dram_parameter("ubp", [128, H], F32, isOutput=False)
    msk_d = nc.declare_dram_parameter("msk", [128, H], F32, isOutput=False)
    cst_d = nc.declare_dram_parameter("csT", [128, H], F32, isOutput=False)
    cct_d = nc.declare_dram_parameter("ccT", [128, H], F32, isOutput=False)
    sn_d = nc.declare_dram_parameter("SN", [128, T], DT, isOutput=False)
    cs_d = nc.declare_dram_parameter("CS", [128, T], DT, isOutput=False)
    out_d = nc.declare_dram_parameter("out", [T, D], F32, isOutput=True)

    Exp = mybir.ActivationFunctionType.Exp
    Copy = mybir.ActivationFunctionType.Copy
    MUL = mybir.AluOpType.mult
    ADD = mybir.AluOpType.add
    SUB = mybir.AluOpType.subtract

    with tile.TileContext(nc) as tc, ExitStack() as ctx:
        cpool = ctx.enter_context(tc.tile_pool(name="consts", bufs=1))
        gpool = ctx.enter_context(tc.tile_pool(name="gwork", bufs=2))
        apool = ctx.enter_context(tc.tile_pool(name="attn", bufs=2))
        opool = ctx.enter_context(tc.tile_pool(name="osb", bufs=4))
        rpool = ctx.enter_context(tc.tile_pool(name="recip", bufs=2))
        ps_s = ctx.enter_context(tc.tile_pool(name="ps_s", bufs=4, space="PSUM"))
        ps_g = ctx.enter_context(tc.tile_pool(name="ps_g", bufs=2, space="PSUM"))
        ps_z = ctx.enter_context(tc.tile_pool(name="ps_z", bufs=2, space="PSUM"))

        # ---- load constants / inputs into SBUF ----
        # one wide tile per tensor, one coalesced DMA (blocks along free dim)
        def load_wide(dram, rows, cols, tag):
            nblk = rows // 128
            t = cpool.tile([128, nblk * cols], DT, tag=tag, name=tag)
            nc.sync.dma_start(
                t[:].rearrange("p (c i) -> p c i", c=nblk),
                dram[:, :].rearrange("(c p) i -> p c i", p=128))
            return [t[:, c * cols:(c + 1) * cols] for c in range(nblk)]

        # PE warm-up during the input-DMA window: 12 dependency-free matmuls
        # all writing ONE psum tile (WAW keeps them in-order on PE; no pool
        # churn), so HAM reaches 8/8 before the first real matmul
        warm = cpool.tile([128, 512], DT, tag="warm", name="warm")
        nc.vector.memset(warm[:], 0.0)
        wp = ps_z.tile([128, 512], F32, tag="z", name="warmp")
        for w in range(12):
            nc.tensor.matmul(wp[:], warm[:, 0:128], warm[:], start=True,
                             stop=True)

        # interleave the first chunks of xsT and Wq so the first projection
        # matmul can issue as early as possible
        xsT_tile = cpool.tile([128, 4 * T], DT, tag="xsT", name="xsT")
        wq_tile = cpool.tile([128, 4 * D], DT, tag="wq", name="wq")
        for c in range(4):
            nc.sync.dma_start(xsT_tile[:, c * T:(c + 1) * T],
                              xsT_d[c * 128:(c + 1) * 128, :])
            nc.sync.dma_start(wq_tile[:, c * D:(c + 1) * D],
                              wq_d[c * 128:(c + 1) * 128, :])
        xsT = [xsT_tile[:, c * T:(c + 1) * T] for c in range(4)]
        wq = [wq_tile[:, c * D:(c + 1) * D] for c in range(4)]
        ubp = cpool.tile([128, H], F32, tag="ubp")
        nc.sync.dma_start(ubp[:], ubp_d[:, :])
        msk = cpool.tile([128, H], F32, tag="msk")
        nc.sync.dma_start(msk[:], msk_d[:, :])
        cst = cpool.tile([128, H], F32, tag="cst")
        nc.sync.dma_start(cst[:], cst_d[:, :])
        cct = cpool.tile([128, H], F32, tag="cct")
        nc.sync.dma_start(cct[:], cct_d[:, :])
        # DMA order tracks first use: K-proj wk, then G~(0) wts/wtc,
        # rope(0)/scores sn/cs, V-proj wv, out-proj wout
        wk = load_wide(wk_d, D, D, "wk")
        wts = cpool.tile([128, H * M], DT, tag="wts", name="wts")
        nc.sync.dma_start(wts[:], wts_d[:, :])
        wtc = cpool.tile([128, H * M], DT, tag="wtc", name="wtc")
        nc.sync.dma_start(wtc[:], wtc_d[:, :])
        sn = cpool.tile([128, T], DT, tag="sn", name="sn")
        nc.sync.dma_start(sn[:], sn_d[:, :])
        cs = cpool.tile([128, T], DT, tag="cs", name="cs")
        nc.sync.dma_start(cs[:], cs_d[:, :])
        wv = load_wide(wv_d, D, D, "wv")
        wout = load_wide(wout_d, D, D, "wout")

        # computed persistent tensors: qup[h] is the per-head zero-padded
        # moving operand (head h's qu rows at (h%2)*64, sibling rows zero) so
        # the score B1 matmul contracts a full 128 rows against the natural
        # head-pair K^T stationary without mixing stationary heights.
        qup = [cpool.tile([128, T], DT, tag=f"qup{h}", name=f"qup{h}")
               for h in range(H)]
        kTn = [cpool.tile([128, T], DT, tag=f"kTn{c}", name=f"kTn{c}")
               for c in range(4)]
        zT = [cpool.tile([128, T], DT, tag=f"zT{c}", name=f"zT{c}")
              for c in range(4)]
        vp = cpool.tile([128, 8 * 520], DT, tag="vp")

        # gpsimd ucode library providing InstPartitionBroadcast
        nc.gpsimd.load_library(library_config.attn)
        # ones columns for the softmax-sum trick (V gets overwritten on top)
        nc.gpsimd.memset(vp[:], 1.0)

        # ---- per-head G~ -> rope(A~) pipeline helpers ----
        def emit_g_mm(h, icnk):
            """G~s/G~c matmuls for head h, one i-chunk. Returns (gs, gc) psum."""
            gs = ps_g.tile([128, 512], F32, tag="g", name="gs")
            nc.tensor.matmul(
                gs[:], wts[:, h * M:(h + 1) * M],
                qup[h][:, icnk * 512:(icnk + 1) * 512],
                start=True, stop=True)
            gc = ps_g.tile([128, 512], F32, tag="g", name="gc")
            nc.tensor.matmul(
                gc[:], wtc[:, h * M:(h + 1) * M],
                qup[h][:, icnk * 512:(icnk + 1) * 512],
                start=True, stop=True)
            return gs, gc

        def emit_rope(h, icnk, gs, gc, m2, m3, tbuf):
            """DVE: const-row add + rotation; writes A~s -> m2, A~c -> m3."""
            sl = slice(icnk * 512, (icnk + 1) * 512)
            gsb = tbuf[:, 0:512]
            gcb = tbuf[:, 512:1024]
            t1 = tbuf[:, 1024:1536]
            t2 = tbuf[:, 1536:2048]
            nc.vector.tensor_scalar_add(gsb, gs[:], cst[:, h:h + 1])
            nc.vector.tensor_scalar_add(gcb, gc[:], cct[:, h:h + 1])
            nc.vector.tensor_tensor(t1, gsb, cs[:, sl], op=MUL)
            nc.vector.tensor_tensor(t2, gcb, sn[:, sl], op=MUL)
            nc.vector.tensor_tensor(m2[:, sl], t1, t2, op=ADD)
            nc.vector.tensor_tensor(t1, gcb, cs[:, sl], op=MUL)
            nc.vector.tensor_tensor(t2, gsb, sn[:, sl], op=MUL)
            nc.vector.tensor_tensor(m3[:, sl], t1, t2, op=SUB)

        # ---- projections ----
        # Q and K psums are interleaved so their evictions drain on DIFFERENT
        # engines in parallel (Q -> DVE masked writes into the per-head
        # zero-padded qup tiles, K -> ACT copies into natural kTn tiles);
        # either alone outpaces PE and stalls the psum rotation.
        def emit_q_half(nchunk, icnk):
            p = ps_s.tile([128, 512], F32, tag="s")
            for kc in range(4):
                nc.tensor.matmul(
                    p[:],
                    wq[kc][:, nchunk * 128:(nchunk + 1) * 128],
                    xsT[kc][:, icnk * 512:(icnk + 1) * 512],
                    start=(kc == 0),
                    stop=(kc == 3),
                )
            for sub in range(2):
                h = 2 * nchunk + sub
                dst = qup[h][:, icnk * 512:(icnk + 1) * 512]
                # (psum * mask_h) + ubias_padded_h: writes the head's 64
                # real rows AND zeroes the sibling rows in one DVE op
                nc.vector.tensor_scalar(
                    dst, p[:], msk[:, h:h + 1], ubp[:, h:h + 1],
                    op0=MUL, op1=ADD)

        def emit_k_half(nchunk, jc):
            p = ps_s.tile([128, 512], F32, tag="s")
            for kc in range(4):
                nc.tensor.matmul(
                    p[:],
                    wk[kc][:, nchunk * 128:(nchunk + 1) * 128],
                    xsT[kc][:, jc * 512:(jc + 1) * 512],
                    start=(kc == 0),
                    stop=(kc == 3),
                )
            nc.scalar.activation(
                kTn[nchunk][:, jc * 512:(jc + 1) * 512], p[:], Copy)

        for nchunk in range(4):
            emit_q_half(nchunk, 0)
            emit_k_half(nchunk, 0)
            emit_q_half(nchunk, 1)
            emit_k_half(nchunk, 1)

        # head 0's G~ after the Q/K projections: rope(0) rides DVE under the
        # V-projection matmuls, so scores(0) never wait on it
        m2_g0 = gpool.tile([128, T], DT, tag="m2", name="m2")
        m3_g0 = gpool.tile([128, T], DT, tag="m3", name="m3")
        tb_g0 = gpool.tile([128, 2048], DT, tag="ropet", name="ropet")
        gs_g0, gc_g0 = emit_g_mm(0, 0)
        emit_rope(0, 0, gs_g0, gc_g0, m2_g0, m3_g0, tb_g0)
        m2_cur, m3_cur = m2_g0, m3_g0

        # V[j,n] = sum_d xsT[d,j] Wv[d,n]; store with stride 65 into vp.
        # G~(0) icnk1 slots in after the fourth V chunk (its ps_g rotation is
        # gated behind rope(0,icnk0) on DVE).
        for jt in range(8):
            if jt == 4:
                gs_g1, gc_g1 = emit_g_mm(0, 1)
                emit_rope(0, 1, gs_g1, gc_g1, m2_g0, m3_g0, tb_g0)
            p = ps_s.tile([128, 512], F32, tag="s")
            for kc in range(4):
                nc.tensor.matmul(
                    p[:],
                    xsT[kc][:, jt * 128:(jt + 1) * 128],
                    wv[kc][:],
                    start=(kc == 0),
                    stop=(kc == 3),
                )
            dst = vp[:, jt * 520:(jt + 1) * 520].rearrange(
                "p (h x) -> p h x", h=8)[:, :, 0:64]
            src = p[:].rearrange("p (h x) -> p h x", h=8)
            nc.scalar.activation(dst, src, Copy)

        def emit_av_mm(h, icnk, jt, zp, attnT):
            nc.tensor.matmul(
                zp[:],
                vp[:, jt * 520 + 65 * h: jt * 520 + 65 * h + 65],
                attnT[:, jt * 1024 + icnk * 512:
                      jt * 1024 + icnk * 512 + 512],
                start=(jt == 0),
                stop=(jt == 7),
            )

        Ln = mybir.ActivationFunctionType.Ln

        def emit_znorm(h, icnk, zp):
            # 1/s computed as exp(-ln s) on ACT (DVE reciprocal is 8 cyc/elem)
            row = (h % 2) * 64
            lns = rpool.tile([1, 512], F32, tag="lns")
            nc.scalar.activation(lns[:], zp[64:65, :], Ln)
            rec = rpool.tile([1, 512], F32, tag="rec")
            nc.scalar.activation(rec[:], lns[:], Exp, scale=-1.0)
            recb = rpool.tile([64, 512], F32, tag="recb")
            nc.gpsimd.partition_broadcast(recb[:], rec[0:1, :])
            dst = zT[h // 2][row:row + 64, icnk * 512:(icnk + 1) * 512]
            nc.vector.tensor_tensor(dst, zp[0:64, :], recb[:], op=MUL)

        for h in range(H):
            attnT = apool.tile([128, 8192], DT, tag="attnT")
            zp0 = ps_z.tile([65, 512], F32, tag="z", name="zp0")
            zp1 = ps_z.tile([65, 512], F32, tag="z", name="zp1")
            if h + 1 < H:
                m2_nx = gpool.tile([128, T], DT, tag="m2", name="m2")
                m3_nx = gpool.tile([128, T], DT, tag="m3", name="m3")
                tb_nx = gpool.tile([128, 2048], DT, tag="ropet", name="ropet")
            # both i-chunks per jt: consecutive matmuls share each stationary
            # operand, halving the LDWEIGHTS issue pressure
            for jt in range(8):
                p0 = ps_s.tile([128, 512], F32, tag="s", name="p0")
                p1 = ps_s.tile([128, 512], F32, tag="s", name="p1")
                for icnk, p in ((0, p0), (1, p1)):
                    nc.tensor.matmul(
                        p[:],
                        kTn[h // 2][:, jt * 128:(jt + 1) * 128],
                        qup[h][:, icnk * 512:(icnk + 1) * 512],
                        start=True,
                        stop=False,
                    )
                for icnk, p in ((0, p0), (1, p1)):
                    nc.tensor.matmul(
                        p[:],
                        sn[:, jt * 128:(jt + 1) * 128],
                        m2_cur[:, icnk * 512:(icnk + 1) * 512],
                        start=False,
                        stop=False,
                    )
                for icnk, p in ((0, p0), (1, p1)):
                    nc.tensor.matmul(
                        p[:],
                        cs[:, jt * 128:(jt + 1) * 128],
                        m3_cur[:, icnk * 512:(icnk + 1) * 512],
                        start=False,
                        stop=True,
                    )
                for icnk, p in ((0, p0), (1, p1)):
                    nc.scalar.activation(
                        attnT[:, jt * 1024 + icnk * 512:
                              jt * 1024 + icnk * 512 + 512],
                        p[:], Exp, scale=float(SCALE))

                # pipeline next head's G~ matmuls + rope inside this head's
                # score loop (PE picks up the 2 small matmuls between score
                # tiles; rope rides on DVE)
                if h + 1 < H:
                    if jt == 0:
                        gs_nx0, gc_nx0 = emit_g_mm(h + 1, 0)
                    elif jt == 1:
                        emit_rope(h + 1, 0, gs_nx0, gc_nx0, m2_nx, m3_nx,
                                  tb_nx)
                    elif jt == 4:
                        gs_nx1, gc_nx1 = emit_g_mm(h + 1, 1)
                    elif jt == 5:
                        emit_rope(h + 1, 1, gs_nx1, gc_nx1, m2_nx, m3_nx,
                                  tb_nx)

            for jt in range(8):
                emit_av_mm(h, 0, jt, zp0, attnT)
            for jt in range(8):
                emit_av_mm(h, 1, jt, zp1, attnT)
            emit_znorm(h, 0, zp0)
            emit_znorm(h, 1, zp1)

            if h + 1 < H:
                m2_cur, m3_cur = m2_nx, m3_nx

        # ---- output projection ----
        # 4 psum tiles in flight per group; the zT[3] (heads 6/7) matmuls are
        # deferred to a second pass so the last head's znorm latency hides
        # behind the ncnk 0..2 accumulation matmuls
        for grp in range(2):
            ps = []
            for it4 in range(4):
                p = ps_s.tile([128, 512], F32, tag="s")
                ps.append(p)
                it = grp * 4 + it4
                for ncnk in range(3):
                    nc.tensor.matmul(
                        p[:],
                        zT[ncnk][:, it * 128:(it + 1) * 128],
                        wout[ncnk][:],
                        start=(ncnk == 0),
                        stop=False,
                    )
            for it4 in range(4):
                it = grp * 4 + it4
                p = ps[it4]
                nc.tensor.matmul(
                    p[:],
                    zT[3][:, it * 128:(it + 1) * 128],
                    wout[3][:],
                    start=False,
                    stop=True,
                )
                osb = opool.tile([128, 512], F32, tag="osb")
                nc.scalar.activation(osb[:], p[:], Copy)
                nc.sync.dma_start(out_d[it * 128:(it + 1) * 128, :], osb[:])

    nc.compile()
    _dedup_ldweights(nc)
    return nc


def _dedup_ldweights(nc):
    """Drop an InstLdweights when the immediately-preceding PE weight load
    (with only matmuls in between) loaded the identical stationary operand.
    Our paired score matmuls reuse each stationary operand twice; the
    duplicate load is what limits the PE instruction issue rate."""
    removed = 0
    for fn in nc.m.functions:
        for blk in fn.blocks:
            last_sig = None
            newlist = []
            for inst in blk.instructions:
                if isinstance(inst, mybir.InstLdweights):
                    sig = str(inst.ins[0])
                    si = inst.sync_info
                    clean = si is None or (
                        len(si.on_wait) == 0 and len(si.on_update) == 0)
                    if clean and sig == last_sig:
                        removed += 1
                        continue
                    last_sig = sig
                    newlist.append(inst)
                else:
                    newlist.append(inst)
            blk.instructions[:] = newlist
    return removed


def make_host_inputs(xs, Wq, Wk, Wv, Wpos, Wout, u_bias, v_bias, mode=MM_MODE):
    """Build the per-core input maps (host-side layout prep only)."""
    npdt = _np_dt(mode)
    nu, As, Ac = _freq_fit()

    ii = np.arange(T, dtype=np.float64)
    SN = np.sin(np.outer(nu, ii)).astype(np.float32)     # (M, T)
    CS = np.cos(np.outer(nu, ii)).astype(np.float32)

    # per-head folded weights: Wt_s_h = Wsin_h.T @ As.T  (64 x M), padded
    # into the 128-row head-pair coordinate (head h rows at (h%2)*64, rest 0)
    perm = np.concatenate([np.arange(0, D, 2), np.arange(1, D, 2)])
    Wpos_perm = np.asarray(Wpos, np.float64)[perm, :]     # (512, H*DH)
    WTS = np.zeros((128, H * M), np.float32)
    WTC = np.zeros((128, H * M), np.float32)
    csT = np.zeros((128, H), np.float32)
    ccT = np.zeros((128, H), np.float32)
    u64 = np.asarray(u_bias, np.float64)
    v64 = np.asarray(v_bias, np.float64)
    for h in range(H):
        Wsin = Wpos_perm[:256, h * DH:(h + 1) * DH]       # (256, 64)
        Wcos = Wpos_perm[256:, h * DH:(h + 1) * DH]
        Wt_s = Wsin.T @ As.T                              # (64, M)
        Wt_c = Wcos.T @ Ac.T
        row = (h % 2) * 64
        WTS[row:row + 64, h * M:(h + 1) * M] = Wt_s
        WTC[row:row + 64, h * M:(h + 1) * M] = Wt_c
        dvu = v64[h] - u64[h]                             # (64,)
        csT[:M, h] = dvu @ Wt_s
        ccT[:M, h] = dvu @ Wt_c

    # per-head zero-padded u_bias columns + row masks in head-pair coords
    ubp = np.zeros((128, H), np.float32)
    mskm = np.zeros((128, H), np.float32)
    for h in range(H):
        row = (h % 2) * 64
        ubp[row:row + 64, h] = np.asarray(u_bias, np.float32)[h]
        mskm[row:row + 64, h] = 1.0

    shared = {
        "Wq": np.ascontiguousarray(Wq).astype(npdt),
        "Wk": np.ascontiguousarray(Wk).astype(npdt),
        "Wv": np.ascontiguousarray(Wv).astype(npdt),
        "WTS": WTS.astype(npdt),
        "WTC": WTC.astype(npdt),
        "Wout": np.ascontiguousarray(Wout).astype(npdt),
        "ubp": ubp,
        "msk": mskm,
        "csT": csT,
        "ccT": ccT,
        "SN": SN.astype(npdt),
        "CS": CS.astype(npdt),
    }
    in_maps = []
    for b in range(B):
        m = dict(shared)
        m["xsT"] = np.ascontiguousarray(xs[b].T).astype(npdt)
        in_maps.append(m)
    return in_maps


_NC_CACHE = {}


def get_nc(mode=MM_MODE):
    if mode not in _NC_CACHE:
        _NC_CACHE[mode] = build_nc(mode)
    return _NC_CACHE[mode]


def _numpy_reference(xs, mask, Wq, Wk, Wv, Wpos, Wout, u_bias, v_bias):
    """Exact (fp32 numpy) fallback for non-all-ones masks."""
    b, t, _ = xs.shape
    pos = np.arange(-(t - 1), t, dtype=np.float32)[:, None]
    inv_freq = np.exp(-np.log(10000.0) *
                      np.arange(0, D, 2, dtype=np.float32) / D)
    angv = pos * inv_freq[None, :]
    pe = np.stack([np.sin(angv), np.cos(angv)], axis=-1).reshape(pos.shape[0], D)
    q = (xs @ Wq).reshape(b, t, H, DH).transpose(0, 2, 1, 3)
    k = (xs @ Wk).reshape(b, t, H, DH).transpose(0, 2, 1, 3)
    v = (xs @ Wv).reshape(b, t, H, DH).transpose(0, 2, 1, 3)
    p = (pe @ Wpos).reshape(-1, H, DH).transpose(1, 0, 2)
    q_u = q + u_bias[None, :, None, :]
    q_v = q + v_bias[None, :, None, :]
    ac = np.einsum("bhtd,bhsd->bhts", q_u, k)
    bd = np.einsum("bhtd,hld->bhtl", q_v, p)
    bdp = np.pad(bd, ((0, 0), (0, 0), (0, 0), (1, 0)))
    l = bd.shape[-1]
    bd = bdp.reshape(b, H, l + 1, t)[:, :, 1:, :].reshape(b, H, t, l)[..., :t]
    scores = (ac + bd) * SCALE
    m = (mask[:, None, :, :] == 0)
    scores = np.where(m, -np.inf, scores)
    scores = scores - scores.max(axis=-1, keepdims=True)
    e = np.exp(scores)
    attn = e / e.sum(axis=-1, keepdims=True)
    attn = np.where(m, 0.0, attn)
    z = np.einsum("bhts,bhsd->bthd", attn, v).reshape(b, t, H * DH)
    return (z @ Wout).astype(np.float32)


def kernel(xs, mask, Wq, Wk, Wv, Wpos, Wout, u_bias, v_bias):
    xs = np.asarray(xs, dtype=np.float32)
    mask = np.asarray(mask)
    Wq = np.asarray(Wq, dtype=np.float32)
    Wk = np.asarray(Wk, dtype=np.float32)
    Wv = np.asarray(Wv, dtype=np.float32)
    Wpos = np.asarray(Wpos, dtype=np.float32)
    Wout = np.asarray(Wout, dtype=np.float32)
    u_bias = np.asarray(u_bias, dtype=np.float32)
    v_bias = np.asarray(v_bias, dtype=np.float32)

    if not np.all(mask != 0):
        # the on-device kernel assumes the (spec-pinned) all-ones mask
        return _numpy_reference(xs, mask, Wq, Wk, Wv, Wpos, Wout, u_bias, v_bias)

    nc = get_nc(MM_MODE)
    in_maps = make_host_inputs(xs, Wq, Wk, Wv, Wpos, Wout, u_bias, v_bias,
                               MM_MODE)
    res = run_bass_kernel_spmd(nc, in_maps, core_ids=list(range(NCORES)))
    out = np.stack([np.asarray(res.results[b]["out"], dtype=np.float32)
                    for b in range(B)], axis=0)
    return out


if __name__ == "__main__":
    # smoke-test: build only
    nc = build_nc()
    print("build ok")


# revision 29
# speedup vs baseline: 1.0356x; 1.0356x over previous
"""Trainium2 Bass kernel for Transformer-XL style relative-position multi-head
self-attention (nn_MultiHeadedSelfAttention_35588099015524).

Sharding: batch (B=8) is data-parallel across the 8 NeuronCores; no collectives.

Math trick (v2 — compressed frequency basis): the Transformer-XL relative
shift term is
    bd[i,j] = qv_i . p_{j-i},   p_d = pe_d @ Wpos   (per head)
with pe the 512-dim sinusoid table over log-spaced frequencies w_c. The 512
basis functions {sin(w_c d), cos(w_c d)} restricted to the window
d in (-1024, 1024) are numerically rank-deficient: a least-squares fit onto
M = 128 frequencies nu_m (the top-96 w_c kept exactly + a 32-point linear
grid under them) reproduces all 512 functions to ~1e-13 (triangular-weighted
over the actual (i,j) usage counts). Writing
    F_e(d) ~= sum_m As[m,e] sin(nu_m d) + Ac[m,e] cos(nu_m d)
and applying the angle-addition formulas gives the exact-rank-256 form
    bd[i,j] = sum_m sin(nu_m j) As~[i,m] + cos(nu_m j) Ac~[i,m]
    As~[i,m] =  cos(nu_m i) Gs[i,m] + sin(nu_m i) Gc[i,m]
    Ac~[i,m] =  cos(nu_m i) Gc[i,m] - sin(nu_m i) Gs[i,m]
    Gs = qv @ (Wpos_h^T As^T),   Gc = qv @ (Wpos_h^T Ac^T)     (64 x 128 each)
so the per-score-tile contraction is 64 (qu.k) + 128 (As~) + 128 (Ac~)
= 3 matmul instructions instead of 5 (the old exact-512 sinusoid form).
Scores are computed transposed (S^T[j,i]) so softmax-normalization sums ride
along as an extra ones-column in V and no on-device transposes are needed.
The v_bias contribution rides as a per-frequency constant row added to Gs/Gc
on DVE ((v-u) @ W~ precomputed on host; G~ itself is computed from qu).
"""

import sys

sys.path.insert(0, "/opt/trn_rl_repo")

from contextlib import ExitStack  # noqa: E402

import numpy as np  # noqa: E402
import ml_dtypes  # noqa: E402

import concourse.bass as bass  # noqa: E402
from concourse import bacc, library_config  # noqa: E402
import concourse.tile as tile  # noqa: E402
from concourse import mybir  # noqa: E402
from concourse.bass_utils import run_bass_kernel_spmd  # noqa: E402

# Force every ACT function we use (Exp/Ln/Copy) to resolve to the single
# "natural_log_exp_and_others" table set — otherwise the table-load pass
# flip-flops between sets per head (~2.7us per ACT_TABLE_LOAD).
import concourse.hw_specs as _hs  # noqa: E402
import concourse.bacc as _bacc_mod  # noqa: E402

if not getattr(_hs, "_act_tables_pinned", False):
    _orig_gat = _hs.get_activation_tables

    def _pinned_gat(arch):
        tabs = _orig_gat(arch)
        keep = "natural_log_exp_and_others"
        pin = {mybir.ActivationFunctionType.Exp,
               mybir.ActivationFunctionType.Ln,
               mybir.ActivationFunctionType.Copy}
        if keep in tabs and pin <= tabs[keep]:
            for k in tabs:
                if k != keep:
                    tabs[k] = tabs[k] - pin
        return tabs

    _hs.get_activation_tables = _pinned_gat
    _bacc_mod.get_activation_tables = _pinned_gat
    _hs._act_tables_pinned = True

B, T, D = 8, 1024, 512
H, DH = 8, 64
NCORES = 8
M = 128                      # compressed frequency count (2M = 256 bd rank)
SCALE = 1.0 / np.sqrt(DH)

F32 = mybir.dt.float32
BF16 = mybir.dt.bfloat16

# knob: matmul/elementwise working dtype ("bf16" or "f32r")
MM_MODE = "bf16"


def _np_dt(mode):
    return ml_dtypes.bfloat16 if mode == "bf16" else np.float32


def _mm_dt(mode):
    return BF16 if mode == "bf16" else mybir.dt.float32r


# ---------------------------------------------------------------------------
# host-side constant precompute: frequency fit (input-independent, cached)
# ---------------------------------------------------------------------------
_FIT_CACHE = {}


def _freq_fit():
    """Least-squares fit of the 512 reference sinusoids onto M frequencies.

    Returns (nu (M,), As (M,256), Ac (M,256)) such that over the triangular-
    weighted window d in (-1024, 1024):
        sin(w_c d) ~= sum_m As[m,c] sin(nu_m d)
        cos(w_c d) ~= sum_m Ac[m,c] cos(nu_m d)
    """
    if "fit" in _FIT_CACHE:
        return _FIT_CACHE["fit"]
    c = np.arange(256)
    omega = np.exp(-np.log(10000.0) * (2.0 * c) / D)
    delta = np.arange(-(T - 1), T, dtype=np.float64)
    w = (T - np.abs(delta)) / T
    sw = np.sqrt(w)[:, None]
    c0, ngrid = M - 32, 32
    nu = np.concatenate([omega[:c0],
                         np.linspace(0, omega[c0 - 1], ngrid, endpoint=False)])
    ang_t = np.outer(delta, omega)
    Bs = np.sin(np.outer(delta, nu)) * sw
    Bc = np.cos(np.outer(delta, nu)) * sw
    As, *_ = np.linalg.lstsq(Bs, np.sin(ang_t) * sw, rcond=None)
    Ac, *_ = np.linalg.lstsq(Bc, np.cos(ang_t) * sw, rcond=None)
    _FIT_CACHE["fit"] = (nu, As, Ac)
    return _FIT_CACHE["fit"]


def build_nc(mode=MM_MODE):
    """Build the per-core Bass module (identical program on all 8 cores)."""
    DT = _mm_dt(mode)
    nc = bacc.Bacc("TRN2", target_bir_lowering=False, debug=False)

    # ---- DRAM parameters (per core) ----
    xsT_d = nc.declare_dram_parameter("xsT", [D, T], DT, isOutput=False)
    wq_d = nc.declare_dram_parameter("Wq", [D, D], DT, isOutput=False)
    wk_d = nc.declare_dram_parameter("Wk", [D, D], DT, isOutput=False)
    wv_d = nc.declare_dram_parameter("Wv", [D, D], DT, isOutput=False)
    wts_d = nc.declare_dram_parameter("WTS", [128, H * M], DT, isOutput=False)
    wtc_d = nc.declare_dram_parameter("WTC", [128, H * M], DT, isOutput=False)
    wout_d = nc.declare_dram_parameter("Wout", [D, D], DT, isOutput=False)
    ubp_d = nc.declare_dram_parameter("ubp", [128, H], F32, isOutput=False)
    msk_d = nc.declare_dram_parameter("msk", [128, H], F32, isOutput=False)
    cst_d = nc.declare_dram_parameter("csT", [128, H], F32, isOutput=False)
    cct_d = nc.declare_dram_parameter("ccT", [128, H], F32, isOutput=False)
    sn_d = nc.declare_dram_parameter("SN", [128, T], DT, isOutput=False)
    cs_d = nc.declare_dram_parameter("CS", [128, T], DT, isOutput=False)
    out_d = nc.declare_dram_parameter("out", [T, D], F32, isOutput=True)

    Exp = mybir.ActivationFunctionType.Exp
    Copy = mybir.ActivationFunctionType.Copy
    MUL = mybir.AluOpType.mult
    ADD = mybir.AluOpType.add
    SUB = mybir.AluOpType.subtract

    with tile.TileContext(nc) as tc, ExitStack() as ctx:
        cpool = ctx.enter_context(tc.tile_pool(name="consts", bufs=1))
        gpool = ctx.enter_context(tc.tile_pool(name="gwork", bufs=2))
        apool = ctx.enter_context(tc.tile_pool(name="attn", bufs=2))
        opool = ctx.enter_context(tc.tile_pool(name="osb", bufs=4))
        rpool = ctx.enter_context(tc.tile_pool(name="recip", bufs=2))
        ps_s = ctx.enter_context(tc.tile_pool(name="ps_s", bufs=4, space="PSUM"))
        ps_g = ctx.enter_context(tc.tile_pool(name="ps_g", bufs=2, space="PSUM"))
        ps_z = ctx.enter_context(tc.tile_pool(name="ps_z", bufs=2, space="PSUM"))

        # ---- load constants / inputs into SBUF ----
        # one wide tile per tensor, one coalesced DMA (blocks along free dim)
        def load_wide(dram, rows, cols, tag):
            nblk = rows // 128
            t = cpool.tile([128, nblk * cols], DT, tag=tag, name=tag)
            nc.sync.dma_start(
                t[:].rearrange("p (c i) -> p c i", c=nblk),
                dram[:, :].rearrange("(c p) i -> p c i", p=128))
            return [t[:, c * cols:(c + 1) * cols] for c in range(nblk)]

        # PE warm-up during the input-DMA window: 12 dependency-free matmuls
        # all writing ONE psum tile (WAW keeps them in-order on PE; no pool
        # churn), so HAM reaches 8/8 before the first real matmul
        warm = cpool.tile([128, 512], DT, tag="warm", name="warm")
        nc.vector.memset(warm[:], 0.0)
        wp = ps_z.tile([128, 512], F32, tag="z", name="warmp")
        for w in range(12):
            nc.tensor.matmul(wp[:], warm[:, 0:128], warm[:], start=True,
                             stop=True)

        # interleave the first chunks of xsT and Wq so the first projection
        # matmul can issue as early as possible
        xsT_tile = cpool.tile([128, 4 * T], DT, tag="xsT", name="xsT")
        wq_tile = cpool.tile([128, 4 * D], DT, tag="wq", name="wq")
        for c in range(4):
            nc.sync.dma_start(xsT_tile[:, c * T:(c + 1) * T],
                              xsT_d[c * 128:(c + 1) * 128, :])
            nc.sync.dma_start(wq_tile[:, c * D:(c + 1) * D],
                              wq_d[c * 128:(c + 1) * 128, :])
        xsT = [xsT_tile[:, c * T:(c + 1) * T] for c in range(4)]
        wq = [wq_tile[:, c * D:(c + 1) * D] for c in range(4)]
        ubp = cpool.tile([128, H], F32, tag="ubp")
        nc.sync.dma_start(ubp[:], ubp_d[:, :])
        msk = cpool.tile([128, H], F32, tag="msk")
        nc.sync.dma_start(msk[:], msk_d[:, :])
        cst = cpool.tile([128, H], F32, tag="cst")
        nc.sync.dma_start(cst[:], cst_d[:, :])
        cct = cpool.tile([128, H], F32, tag="cct")
        nc.sync.dma_start(cct[:], cct_d[:, :])
        # DMA order tracks first use: K-proj wk, then G~(0) wts/wtc,
        # rope(0)/scores sn/cs, V-proj wv, out-proj wout
        wk = load_wide(wk_d, D, D, "wk")
        wts = cpool.tile([128, H * M], DT, tag="wts", name="wts")
        nc.sync.dma_start(wts[:], wts_d[:, :])
        wtc = cpool.tile([128, H * M], DT, tag="wtc", name="wtc")
        nc.sync.dma_start(wtc[:], wtc_d[:, :])
        sn = cpool.tile([128, T], DT, tag="sn", name="sn")
        nc.sync.dma_start(sn[:], sn_d[:, :])
        cs = cpool.tile([128, T], DT, tag="cs", name="cs")
        nc.sync.dma_start(cs[:], cs_d[:, :])
        wv = load_wide(wv_d, D, D, "wv")
        wout = load_wide(wout_d, D, D, "wout")

        # computed persistent tensors: qup[h] is the per-head zero-padded
        # moving operand (head h's qu rows at (h%2)*64, sibling rows zero) so
        # the score B1 matmul contracts a full 128 rows against the natural
        # head-pair K^T stationary without mixing stationary heights.
        qup = [cpool.tile([128, T], DT, tag=f"qup{h}", name=f"qup{h}")
               for h in range(H)]
        kTn = [cpool.tile([128, T], DT, tag=f"kTn{c}", name=f"kTn{c}")
               for c in range(4)]
        zT = [cpool.tile([128, T], DT, tag=f"zT{c}", name=f"zT{c}")
              for c in range(4)]
        vp = cpool.tile([128, 8 * 520], DT, tag="vp")

        # gpsimd ucode library providing InstPartitionBroadcast
        nc.gpsimd.load_library(library_config.attn)
        # ones columns for the softmax-sum trick (V gets overwritten on top)
        nc.gpsimd.memset(vp[:], 1.0)

        # ---- per-head G~ -> rope(A~) pipeline helpers ----
        def emit_g_mm(h, icnk):
            """G~s/G~c matmuls for head h, one i-chunk. Returns (gs, gc) psum."""
            gs = ps_g.tile([128, 512], F32, tag="g", name="gs")
            nc.tensor.matmul(
                gs[:], wts[:, h * M:(h + 1) * M],
                qup[h][:, icnk * 512:(icnk + 1) * 512],
                start=True, stop=True)
            gc = ps_g.tile([128, 512], F32, tag="g", name="gc")
            nc.tensor.matmul(
                gc[:], wtc[:, h * M:(h + 1) * M],
                qup[h][:, icnk * 512:(icnk + 1) * 512],
                start=True, stop=True)
            return gs, gc

        def emit_rope(h, icnk, gs, gc, m2, m3, tbuf):
            """DVE: const-row add + rotation; writes A~s -> m2, A~c -> m3."""
            sl = slice(icnk * 512, (icnk + 1) * 512)
            gsb = tbuf[:, 0:512]
            gcb = tbuf[:, 512:1024]
            t1 = tbuf[:, 1024:1536]
            t2 = tbuf[:, 1536:2048]
            nc.vector.tensor_scalar_add(gsb, gs[:], cst[:, h:h + 1])
            nc.vector.tensor_scalar_add(gcb, gc[:], cct[:, h:h + 1])
            nc.vector.tensor_tensor(t1, gsb, cs[:, sl], op=MUL)
            nc.vector.tensor_tensor(t2, gcb, sn[:, sl], op=MUL)
            nc.vector.tensor_tensor(m2[:, sl], t1, t2, op=ADD)
            nc.vector.tensor_tensor(t1, gcb, cs[:, sl], op=MUL)
            nc.vector.tensor_tensor(t2, gsb, sn[:, sl], op=MUL)
            nc.vector.tensor_tensor(m3[:, sl], t1, t2, op=SUB)

        # ---- projections ----
        # Q and K psums are interleaved so their evictions drain on DIFFERENT
        # engines in parallel (Q -> DVE masked writes into the per-head
        # zero-padded qup tiles, K -> ACT copies into natural kTn tiles);
        # either alone outpaces PE and stalls the psum rotation.
        def emit_q_half(nchunk, icnk):
            p = ps_s.tile([128, 512], F32, tag="s")
            for kc in range(4):
                nc.tensor.matmul(
                    p[:],
                    wq[kc][:, nchunk * 128:(nchunk + 1) * 128],
                    xsT[kc][:, icnk * 512:(icnk + 1) * 512],
                    start=(kc == 0),
                    stop=(kc == 3),
                )
            for sub in range(2):
                h = 2 * nchunk + sub
                dst = qup[h][:, icnk * 512:(icnk + 1) * 512]
                # (psum * mask_h) + ubias_padded_h: writes the head's 64
                # real rows AND zeroes the sibling rows in one DVE op
                nc.vector.tensor_scalar(
                    dst, p[:], msk[:, h:h + 1], ubp[:, h:h + 1],
                    op0=MUL, op1=ADD)

        def emit_k_half(nchunk, jc):
            # K psums live in ps_g (idle during projections) so the Q
            # eviction backlog on ps_s never blocks the K matmuls
            p = ps_g.tile([128, 512], F32, tag="g")
            for kc in range(4):
                nc.tensor.matmul(
                    p[:],
                    wk[kc][:, nchunk * 128:(nchunk + 1) * 128],
                    xsT[kc][:, jc * 512:(jc + 1) * 512],
                    start=(kc == 0),
                    stop=(kc == 3),
                )
            nc.scalar.activation(
                kTn[nchunk][:, jc * 512:(jc + 1) * 512], p[:], Copy)

        for nchunk in range(4):
            emit_q_half(nchunk, 0)
            emit_q_half(nchunk, 1)
        for nchunk in range(4):
            emit_k_half(nchunk, 0)
            emit_k_half(nchunk, 1)

        # head 0's G~ after the Q/K projections: rope(0) rides DVE under the
        # V-projection matmuls, so scores(0) never wait on it
        m2_g0 = gpool.tile([128, T], DT, tag="m2", name="m2")
        m3_g0 = gpool.tile([128, T], DT, tag="m3", name="m3")
        tb_g0 = gpool.tile([128, 2048], DT, tag="ropet", name="ropet")
        gs_g0, gc_g0 = emit_g_mm(0, 0)
        emit_rope(0, 0, gs_g0, gc_g0, m2_g0, m3_g0, tb_g0)
        m2_cur, m3_cur = m2_g0, m3_g0

        # V[j,n] = sum_d xsT[d,j] Wv[d,n]; store with stride 65 into vp.
        # G~(0) icnk1 slots in after the fourth V chunk (its ps_g rotation is
        # gated behind rope(0,icnk0) on DVE).
        for jt in range(8):
            if jt == 4:
                gs_g1, gc_g1 = emit_g_mm(0, 1)
                emit_rope(0, 1, gs_g1, gc_g1, m2_g0, m3_g0, tb_g0)
            p = ps_s.tile([128, 512], F32, tag="s")
            for kc in range(4):
                nc.tensor.matmul(
                    p[:],
                    xsT[kc][:, jt * 128:(jt + 1) * 128],
                    wv[kc][:],
                    start=(kc == 0),
                    stop=(kc == 3),
                )
            dst = vp[:, jt * 520:(jt + 1) * 520].rearrange(
                "p (h x) -> p h x", h=8)[:, :, 0:64]
            src = p[:].rearrange("p (h x) -> p h x", h=8)
            nc.scalar.activation(dst, src, Copy)

        def emit_av_mm(h, icnk, jt, zp, attnT):
            nc.tensor.matmul(
                zp[:],
                vp[:, jt * 520 + 65 * h: jt * 520 + 65 * h + 65],
                attnT[:, jt * 1024 + icnk * 512:
                      jt * 1024 + icnk * 512 + 512],
                start=(jt == 0),
                stop=(jt == 7),
            )

        Ln = mybir.ActivationFunctionType.Ln

        def emit_znorm(h, icnk, zp):
            # 1/s computed as exp(-ln s) on ACT (DVE reciprocal is 8 cyc/elem)
            row = (h % 2) * 64
            lns = rpool.tile([1, 512], F32, tag="lns")
            nc.scalar.activation(lns[:], zp[64:65, :], Ln)
            rec = rpool.tile([1, 512], F32, tag="rec")
            nc.scalar.activation(rec[:], lns[:], Exp, scale=-1.0)
            recb = rpool.tile([64, 512], F32, tag="recb")
            nc.gpsimd.partition_broadcast(recb[:], rec[0:1, :])
            dst = zT[h // 2][row:row + 64, icnk * 512:(icnk + 1) * 512]
            nc.vector.tensor_tensor(dst, zp[0:64, :], recb[:], op=MUL)

        for h in range(H):
            attnT = apool.tile([128, 8192], DT, tag="attnT")
            zp0 = ps_z.tile([65, 512], F32, tag="z", name="zp0")
            zp1 = ps_z.tile([65, 512], F32, tag="z", name="zp1")
            if h + 1 < H:
                m2_nx = gpool.tile([128, T], DT, tag="m2", name="m2")
                m3_nx = gpool.tile([128, T], DT, tag="m3", name="m3")
                tb_nx = gpool.tile([128, 2048], DT, tag="ropet", name="ropet")
            # both i-chunks per jt: consecutive matmuls share each stationary
            # operand, halving the LDWEIGHTS issue pressure
            for jt in range(8):
                p0 = ps_s.tile([128, 512], F32, tag="s", name="p0")
                p1 = ps_s.tile([128, 512], F32, tag="s", name="p1")
                for icnk, p in ((0, p0), (1, p1)):
                    nc.tensor.matmul(
                        p[:],
                        kTn[h // 2][:, jt * 128:(jt + 1) * 128],
                        qup[h][:, icnk * 512:(icnk + 1) * 512],
                        start=True,
                        stop=False,
                    )
                for icnk, p in ((0, p0), (1, p1)):
                    nc.tensor.matmul(
                        p[:],
                        sn[:, jt * 128:(jt + 1) * 128],
                        m2_cur[:, icnk * 512:(icnk + 1) * 512],
                        start=False,
                        stop=False,
                    )
                for icnk, p in ((0, p0), (1, p1)):
                    nc.tensor.matmul(
                        p[:],
                        cs[:, jt * 128:(jt + 1) * 128],
                        m3_cur[:, icnk * 512:(icnk + 1) * 512],
                        start=False,
                        stop=True,
                    )
                for icnk, p in ((0, p0), (1, p1)):
                    nc.scalar.activation(
                        attnT[:, jt * 1024 + icnk * 512:
                              jt * 1024 + icnk * 512 + 512],
                        p[:], Exp, scale=float(SCALE))

                # pipeline next head's G~ matmuls + rope inside this head's
                # score loop (PE picks up the 2 small matmuls between score
                # tiles; rope rides on DVE)
                if h + 1 < H:
                    if jt == 0:
                        gs_nx0, gc_nx0 = emit_g_mm(h + 1, 0)
                    elif jt == 1:
                        emit_rope(h + 1, 0, gs_nx0, gc_nx0, m2_nx, m3_nx,
                                  tb_nx)
                    elif jt == 4:
                        gs_nx1, gc_nx1 = emit_g_mm(h + 1, 1)
                    elif jt == 5:
                        emit_rope(h + 1, 1, gs_nx1, gc_nx1, m2_nx, m3_nx,
                                  tb_nx)

            for jt in range(8):
                emit_av_mm(h, 0, jt, zp0, attnT)
            for jt in range(8):
                emit_av_mm(h, 1, jt, zp1, attnT)
            emit_znorm(h, 0, zp0)
            emit_znorm(h, 1, zp1)

            if h + 1 < H:
                m2_cur, m3_cur = m2_nx, m3_nx

        # ---- output projection ----
        # 4 psum tiles in flight per group; the zT[3] (heads 6/7) matmuls are
        # deferred to a second pass so the last head's znorm latency hides
        # behind the ncnk 0..2 accumulation matmuls
        for grp in range(2):
            ps = []
            for it4 in range(4):
                p = ps_s.tile([128, 512], F32, tag="s")
                ps.append(p)
                it = grp * 4 + it4
                for ncnk in range(3):
                    nc.tensor.matmul(
                        p[:],
                        zT[ncnk][:, it * 128:(it + 1) * 128],
                        wout[ncnk][:],
                        start=(ncnk == 0),
                        stop=False,
                    )
            for it4 in range(4):
                it = grp * 4 + it4
                p = ps[it4]
                nc.tensor.matmul(
                    p[:],
                    zT[3][:, it * 128:(it + 1) * 128],
                    wout[3][:],
                    start=False,
                    stop=True,
                )
                osb = opool.tile([128, 512], F32, tag="osb")
                nc.scalar.activation(osb[:], p[:], Copy)
                nc.sync.dma_start(out_d[it * 128:(it + 1) * 128, :], osb[:])

    nc.compile()
    _dedup_ldweights(nc)
    return nc


def _dedup_ldweights(nc):
    """Drop an InstLdweights when the immediately-preceding PE weight load
    (with only matmuls in between) loaded the identical stationary operand.
    Our paired score matmuls reuse each stationary operand twice; the
    duplicate load is what limits the PE instruction issue rate."""
    removed = 0
    for fn in nc.m.functions:
        for blk in fn.blocks:
            last_sig = None
            newlist = []
            for inst in blk.instructions:
                if isinstance(inst, mybir.InstLdweights):
                    sig = str(inst.ins[0])
                    si = inst.sync_info
                    clean = si is None or (
                        len(si.on_wait) == 0 and len(si.on_update) == 0)
                    if clean and sig == last_sig:
                        removed += 1
                        continue
                    last_sig = sig
                    newlist.append(inst)
                else:
                    newlist.append(inst)
            blk.instructions[:] = newlist
    return removed


def make_host_inputs(xs, Wq, Wk, Wv, Wpos, Wout, u_bias, v_bias, mode=MM_MODE):
    """Build the per-core input maps (host-side layout prep only)."""
    npdt = _np_dt(mode)
    nu, As, Ac = _freq_fit()

    ii = np.arange(T, dtype=np.float64)
    SN = np.sin(np.outer(nu, ii)).astype(np.float32)     # (M, T)
    CS = np.cos(np.outer(nu, ii)).astype(np.float32)

    # per-head folded weights: Wt_s_h = Wsin_h.T @ As.T  (64 x M), padded
    # into the 128-row head-pair coordinate (head h rows at (h%2)*64, rest 0)
    perm = np.concatenate([np.arange(0, D, 2), np.arange(1, D, 2)])
    Wpos_perm = np.asarray(Wpos, np.float64)[perm, :]     # (512, H*DH)
    WTS = np.zeros((128, H * M), np.float32)
    WTC = np.zeros((128, H * M), np.float32)
    csT = np.zeros((128, H), np.float32)
    ccT = np.zeros((128, H), np.float32)
    u64 = np.asarray(u_bias, np.float64)
    v64 = np.asarray(v_bias, np.float64)
    for h in range(H):
        Wsin = Wpos_perm[:256, h * DH:(h + 1) * DH]       # (256, 64)
        Wcos = Wpos_perm[256:, h * DH:(h + 1) * DH]
        Wt_s = Wsin.T @ As.T                              # (64, M)
        Wt_c = Wcos.T @ Ac.T
        row = (h % 2) * 64
        WTS[row:row + 64, h * M:(h + 1) * M] = Wt_s
        WTC[row:row + 64, h * M:(h + 1) * M] = Wt_c
        dvu = v64[h] - u64[h]                             # (64,)
        csT[:M, h] = dvu @ Wt_s
        ccT[:M, h] = dvu @ Wt_c

    # per-head zero-padded u_bias columns + row masks in head-pair coords
    ubp = np.zeros((128, H), np.float32)
    mskm = np.zeros((128, H), np.float32)
    for h in range(H):
        row = (h % 2) * 64
        ubp[row:row + 64, h] = np.asarray(u_bias, np.float32)[h]
        mskm[row:row + 64, h] = 1.0

    shared = {
        "Wq": np.ascontiguousarray(Wq).astype(npdt),
        "Wk": np.ascontiguousarray(Wk).astype(npdt),
        "Wv": np.ascontiguousarray(Wv).astype(npdt),
        "WTS": WTS.astype(npdt),
        "WTC": WTC.astype(npdt),
        "Wout": np.ascontiguousarray(Wout).astype(npdt),
        "ubp": ubp,
        "msk": mskm,
        "csT": csT,
        "ccT": ccT,
        "SN": SN.astype(npdt),
        "CS": CS.astype(npdt),
    }
    in_maps = []
    for b in range(B):
        m = dict(shared)
        m["xsT"] = np.ascontiguousarray(xs[b].T).astype(npdt)
        in_maps.append(m)
    return in_maps


_NC_CACHE = {}


def get_nc(mode=MM_MODE):
    if mode not in _NC_CACHE:
        _NC_CACHE[mode] = build_nc(mode)
    return _NC_CACHE[mode]


def _numpy_reference(xs, mask, Wq, Wk, Wv, Wpos, Wout, u_bias, v_bias):
    """Exact (fp32 numpy) fallback for non-all-ones masks."""
    b, t, _ = xs.shape
    pos = np.arange(-(t - 1), t, dtype=np.float32)[:, None]
    inv_freq = np.exp(-np.log(10000.0) *
                      np.arange(0, D, 2, dtype=np.float32) / D)
    angv = pos * inv_freq[None, :]
    pe = np.stack([np.sin(angv), np.cos(angv)], axis=-1).reshape(pos.shape[0], D)
    q = (xs @ Wq).reshape(b, t, H, DH).transpose(0, 2, 1, 3)
    k = (xs @ Wk).reshape(b, t, H, DH).transpose(0, 2, 1, 3)
    v = (xs @ Wv).reshape(b, t, H, DH).transpose(0, 2, 1, 3)
    p = (pe @ Wpos).reshape(-1, H, DH).transpose(1, 0, 2)
    q_u = q + u_bias[None, :, None, :]
    q_v = q + v_bias[None, :, None, :]
    ac = np.einsum("bhtd,bhsd->bhts", q_u, k)
    bd = np.einsum("bhtd,hld->bhtl", q_v, p)
    bdp = np.pad(bd, ((0, 0), (0, 0), (0, 0), (1, 0)))
    l = bd.shape[-1]
    bd = bdp.reshape(b, H, l + 1, t)[:, :, 1:, :].reshape(b, H, t, l)[..., :t]
    scores = (ac + bd) * SCALE
    m = (mask[:, None, :, :] == 0)
    scores = np.where(m, -np.inf, scores)
    scores = scores - scores.max(axis=-1, keepdims=True)
    e = np.exp(scores)
    attn = e / e.sum(axis=-1, keepdims=True)
    attn = np.where(m, 0.0, attn)
    z = np.einsum("bhts,bhsd->bthd", attn, v).reshape(b, t, H * DH)
    return (z @ Wout).astype(np.float32)


def kernel(xs, mask, Wq, Wk, Wv, Wpos, Wout, u_bias, v_bias):
    xs = np.asarray(xs, dtype=np.float32)
    mask = np.asarray(mask)
    Wq = np.asarray(Wq, dtype=np.float32)
    Wk = np.asarray(Wk, dtype=np.float32)
    Wv = np.asarray(Wv, dtype=np.float32)
    Wpos = np.asarray(Wpos, dtype=np.float32)
    Wout = np.asarray(Wout, dtype=np.float32)
    u_bias = np.asarray(u_bias, dtype=np.float32)
    v_bias = np.asarray(v_bias, dtype=np.float32)

    if not np.all(mask != 0):
        # the on-device kernel assumes the (spec-pinned) all-ones mask
        return _numpy_reference(xs, mask, Wq, Wk, Wv, Wpos, Wout, u_bias, v_bias)

    nc = get_nc(MM_MODE)
    in_maps = make_host_inputs(xs, Wq, Wk, Wv, Wpos, Wout, u_bias, v_bias,
                               MM_MODE)
    res = run_bass_kernel_spmd(nc, in_maps, core_ids=list(range(NCORES)))
    out = np.stack([np.asarray(res.results[b]["out"], dtype=np.float32)
                    for b in range(B)], axis=0)
    return out


if __name__ == "__main__":
    # smoke-test: build only
    nc = build_nc()
    print("build ok")


# revision 31
# speedup vs baseline: 1.0364x; 1.0008x over previous
"""Trainium2 Bass kernel for Transformer-XL style relative-position multi-head
self-attention (nn_MultiHeadedSelfAttention_35588099015524).

Sharding: batch (B=8) is data-parallel across the 8 NeuronCores; no collectives.

Math trick (v2 — compressed frequency basis): the Transformer-XL relative
shift term is
    bd[i,j] = qv_i . p_{j-i},   p_d = pe_d @ Wpos   (per head)
with pe the 512-dim sinusoid table over log-spaced frequencies w_c. The 512
basis functions {sin(w_c d), cos(w_c d)} restricted to the window
d in (-1024, 1024) are numerically rank-deficient: a least-squares fit onto
M = 128 frequencies nu_m (the top-96 w_c kept exactly + a 32-point linear
grid under them) reproduces all 512 functions to ~1e-13 (triangular-weighted
over the actual (i,j) usage counts). Writing
    F_e(d) ~= sum_m As[m,e] sin(nu_m d) + Ac[m,e] cos(nu_m d)
and applying the angle-addition formulas gives the exact-rank-256 form
    bd[i,j] = sum_m sin(nu_m j) As~[i,m] + cos(nu_m j) Ac~[i,m]
    As~[i,m] =  cos(nu_m i) Gs[i,m] + sin(nu_m i) Gc[i,m]
    Ac~[i,m] =  cos(nu_m i) Gc[i,m] - sin(nu_m i) Gs[i,m]
    Gs = qv @ (Wpos_h^T As^T),   Gc = qv @ (Wpos_h^T Ac^T)     (64 x 128 each)
so the per-score-tile contraction is 64 (qu.k) + 128 (As~) + 128 (Ac~)
= 3 matmul instructions instead of 5 (the old exact-512 sinusoid form).
Scores are computed transposed (S^T[j,i]) so softmax-normalization sums ride
along as an extra ones-column in V and no on-device transposes are needed.
The v_bias contribution rides as a per-frequency constant row added to Gs/Gc
on DVE ((v-u) @ W~ precomputed on host; G~ itself is computed from qu).
"""

import sys

sys.path.insert(0, "/opt/trn_rl_repo")

from contextlib import ExitStack  # noqa: E402

import numpy as np  # noqa: E402
import ml_dtypes  # noqa: E402

import concourse.bass as bass  # noqa: E402
from concourse import bacc, library_config  # noqa: E402
import concourse.tile as tile  # noqa: E402
from concourse import mybir  # noqa: E402
from concourse.bass_utils import run_bass_kernel_spmd  # noqa: E402

# Force every ACT function we use (Exp/Ln/Copy) to resolve to the single
# "natural_log_exp_and_others" table set — otherwise the table-load pass
# flip-flops between sets per head (~2.7us per ACT_TABLE_LOAD).
import concourse.hw_specs as _hs  # noqa: E402
import concourse.bacc as _bacc_mod  # noqa: E402

if not getattr(_hs, "_act_tables_pinned", False):
    _orig_gat = _hs.get_activation_tables

    def _pinned_gat(arch):
        tabs = _orig_gat(arch)
        keep = "natural_log_exp_and_others"
        pin = {mybir.ActivationFunctionType.Exp,
               mybir.ActivationFunctionType.Ln,
               mybir.ActivationFunctionType.Copy}
        if keep in tabs and pin <= tabs[keep]:
            for k in tabs:
                if k != keep:
                    tabs[k] = tabs[k] - pin
        return tabs

    _hs.get_activation_tables = _pinned_gat
    _bacc_mod.get_activation_tables = _pinned_gat
    _hs._act_tables_pinned = True

B, T, D = 8, 1024, 512
H, DH = 8, 64
NCORES = 8
M = 128                      # compressed frequency count (2M = 256 bd rank)
SCALE = 1.0 / np.sqrt(DH)

F32 = mybir.dt.float32
BF16 = mybir.dt.bfloat16

# knob: matmul/elementwise working dtype ("bf16" or "f32r")
MM_MODE = "bf16"


def _np_dt(mode):
    return ml_dtypes.bfloat16 if mode == "bf16" else np.float32


def _mm_dt(mode):
    return BF16 if mode == "bf16" else mybir.dt.float32r


# ---------------------------------------------------------------------------
# host-side constant precompute: frequency fit (input-independent, cached)
# ---------------------------------------------------------------------------
_FIT_CACHE = {}


def _freq_fit():
    """Least-squares fit of the 512 reference sinusoids onto M frequencies.

    Returns (nu (M,), As (M,256), Ac (M,256)) such that over the triangular-
    weighted window d in (-1024, 1024):
        sin(w_c d) ~= sum_m As[m,c] sin(nu_m d)
        cos(w_c d) ~= sum_m Ac[m,c] cos(nu_m d)
    """
    if "fit" in _FIT_CACHE:
        return _FIT_CACHE["fit"]
    c = np.arange(256)
    omega = np.exp(-np.log(10000.0) * (2.0 * c) / D)
    delta = np.arange(-(T - 1), T, dtype=np.float64)
    w = (T - np.abs(delta)) / T
    sw = np.sqrt(w)[:, None]
    c0, ngrid = M - 32, 32
    nu = np.concatenate([omega[:c0],
                         np.linspace(0, omega[c0 - 1], ngrid, endpoint=False)])
    ang_t = np.outer(delta, omega)
    Bs = np.sin(np.outer(delta, nu)) * sw
    Bc = np.cos(np.outer(delta, nu)) * sw
    As, *_ = np.linalg.lstsq(Bs, np.sin(ang_t) * sw, rcond=None)
    Ac, *_ = np.linalg.lstsq(Bc, np.cos(ang_t) * sw, rcond=None)
    _FIT_CACHE["fit"] = (nu, As, Ac)
    return _FIT_CACHE["fit"]


def build_nc(mode=MM_MODE):
    """Build the per-core Bass module (identical program on all 8 cores)."""
    DT = _mm_dt(mode)
    nc = bacc.Bacc("TRN2", target_bir_lowering=False, debug=False)

    # ---- DRAM parameters (per core) ----
    xsT_d = nc.declare_dram_parameter("xsT", [D, T], DT, isOutput=False)
    wq_d = nc.declare_dram_parameter("Wq", [D, D], DT, isOutput=False)
    wk_d = nc.declare_dram_parameter("Wk", [D, D], DT, isOutput=False)
    wv_d = nc.declare_dram_parameter("Wv", [D, D], DT, isOutput=False)
    wts_d = nc.declare_dram_parameter("WTS", [128, H * M], DT, isOutput=False)
    wtc_d = nc.declare_dram_parameter("WTC", [128, H * M], DT, isOutput=False)
    wout_d = nc.declare_dram_parameter("Wout", [D, D], DT, isOutput=False)
    ubp_d = nc.declare_dram_parameter("ubp", [128, H], F32, isOutput=False)
    msk_d = nc.declare_dram_parameter("msk", [128, H], F32, isOutput=False)
    cst_d = nc.declare_dram_parameter("csT", [128, H], F32, isOutput=False)
    cct_d = nc.declare_dram_parameter("ccT", [128, H], F32, isOutput=False)
    sn_d = nc.declare_dram_parameter("SN", [128, T], DT, isOutput=False)
    cs_d = nc.declare_dram_parameter("CS", [128, T], DT, isOutput=False)
    out_d = nc.declare_dram_parameter("out", [T, D], F32, isOutput=True)

    Exp = mybir.ActivationFunctionType.Exp
    Copy = mybir.ActivationFunctionType.Copy
    MUL = mybir.AluOpType.mult
    ADD = mybir.AluOpType.add
    SUB = mybir.AluOpType.subtract

    with tile.TileContext(nc) as tc, ExitStack() as ctx:
        cpool = ctx.enter_context(tc.tile_pool(name="consts", bufs=1))
        gpool = ctx.enter_context(tc.tile_pool(name="gwork", bufs=2))
        apool = ctx.enter_context(tc.tile_pool(name="attn", bufs=2))
        opool = ctx.enter_context(tc.tile_pool(name="osb", bufs=4))
        rpool = ctx.enter_context(tc.tile_pool(name="recip", bufs=2))
        ps_s = ctx.enter_context(tc.tile_pool(name="ps_s", bufs=4, space="PSUM"))
        ps_g = ctx.enter_context(tc.tile_pool(name="ps_g", bufs=2, space="PSUM"))
        ps_z = ctx.enter_context(tc.tile_pool(name="ps_z", bufs=2, space="PSUM"))

        # ---- load constants / inputs into SBUF ----
        # one wide tile per tensor, one coalesced DMA (blocks along free dim)
        def load_wide(dram, rows, cols, tag):
            nblk = rows // 128
            t = cpool.tile([128, nblk * cols], DT, tag=tag, name=tag)
            nc.sync.dma_start(
                t[:].rearrange("p (c i) -> p c i", c=nblk),
                dram[:, :].rearrange("(c p) i -> p c i", p=128))
            return [t[:, c * cols:(c + 1) * cols] for c in range(nblk)]

        # PE warm-up during the input-DMA window: 12 dependency-free matmuls
        # all writing ONE psum tile (WAW keeps them in-order on PE; no pool
        # churn), so HAM reaches 8/8 before the first real matmul
        warm = cpool.tile([128, 512], DT, tag="warm", name="warm")
        nc.vector.memset(warm[:], 0.0)
        wp = ps_z.tile([128, 512], F32, tag="z", name="warmp")
        for w in range(12):
            nc.tensor.matmul(wp[:], warm[:, 0:128], warm[:], start=True,
                             stop=True)

        # interleave the first chunks of xsT and Wq so the first projection
        # matmul can issue as early as possible
        xsT_tile = cpool.tile([128, 4 * T], DT, tag="xsT", name="xsT")
        wq_tile = cpool.tile([128, 4 * D], DT, tag="wq", name="wq")
        for c in range(4):
            nc.sync.dma_start(xsT_tile[:, c * T:(c + 1) * T],
                              xsT_d[c * 128:(c + 1) * 128, :])
            nc.sync.dma_start(wq_tile[:, c * D:(c + 1) * D],
                              wq_d[c * 128:(c + 1) * 128, :])
        xsT = [xsT_tile[:, c * T:(c + 1) * T] for c in range(4)]
        wq = [wq_tile[:, c * D:(c + 1) * D] for c in range(4)]
        ubp = cpool.tile([128, H], F32, tag="ubp")
        nc.sync.dma_start(ubp[:], ubp_d[:, :])
        msk = cpool.tile([128, H], F32, tag="msk")
        nc.sync.dma_start(msk[:], msk_d[:, :])
        cst = cpool.tile([128, H], F32, tag="cst")
        nc.sync.dma_start(cst[:], cst_d[:, :])
        cct = cpool.tile([128, H], F32, tag="cct")
        nc.sync.dma_start(cct[:], cct_d[:, :])
        # DMA order tracks first use: K-proj wk, then G~(0) wts/wtc,
        # rope(0)/scores sn/cs, V-proj wv, out-proj wout
        wv = load_wide(wv_d, D, D, "wv")
        wk = load_wide(wk_d, D, D, "wk")
        wts = cpool.tile([128, H * M], DT, tag="wts", name="wts")
        nc.sync.dma_start(wts[:], wts_d[:, :])
        wtc = cpool.tile([128, H * M], DT, tag="wtc", name="wtc")
        nc.sync.dma_start(wtc[:], wtc_d[:, :])
        sn = cpool.tile([128, T], DT, tag="sn", name="sn")
        nc.sync.dma_start(sn[:], sn_d[:, :])
        cs = cpool.tile([128, T], DT, tag="cs", name="cs")
        nc.sync.dma_start(cs[:], cs_d[:, :])
        wout = load_wide(wout_d, D, D, "wout")

        # computed persistent tensors: qup[h] is the per-head zero-padded
        # moving operand (head h's qu rows at (h%2)*64, sibling rows zero) so
        # the score B1 matmul contracts a full 128 rows against the natural
        # head-pair K^T stationary without mixing stationary heights.
        qup = [cpool.tile([128, T], DT, tag=f"qup{h}", name=f"qup{h}")
               for h in range(H)]
        kTn = [cpool.tile([128, T], DT, tag=f"kTn{c}", name=f"kTn{c}")
               for c in range(4)]
        zT = [cpool.tile([128, T], DT, tag=f"zT{c}", name=f"zT{c}")
              for c in range(4)]
        vp = cpool.tile([128, 8 * 520], DT, tag="vp")

        # gpsimd ucode library providing InstPartitionBroadcast
        nc.gpsimd.load_library(library_config.attn)
        # ones columns for the softmax-sum trick (V gets overwritten on top)
        nc.gpsimd.memset(vp[:], 1.0)

        # ---- per-head G~ -> rope(A~) pipeline helpers ----
        def emit_g_mm(h, icnk):
            """G~s/G~c matmuls for head h, one i-chunk. Returns (gs, gc) psum."""
            gs = ps_g.tile([128, 512], F32, tag="g", name="gs")
            nc.tensor.matmul(
                gs[:], wts[:, h * M:(h + 1) * M],
                qup[h][:, icnk * 512:(icnk + 1) * 512],
                start=True, stop=True)
            gc = ps_g.tile([128, 512], F32, tag="g", name="gc")
            nc.tensor.matmul(
                gc[:], wtc[:, h * M:(h + 1) * M],
                qup[h][:, icnk * 512:(icnk + 1) * 512],
                start=True, stop=True)
            return gs, gc

        def emit_rope(h, icnk, gs, gc, m2, m3, tbuf):
            """DVE: const-row add + rotation; writes A~s -> m2, A~c -> m3."""
            sl = slice(icnk * 512, (icnk + 1) * 512)
            gsb = tbuf[:, 0:512]
            gcb = tbuf[:, 512:1024]
            t1 = tbuf[:, 1024:1536]
            t2 = tbuf[:, 1536:2048]
            nc.vector.tensor_scalar_add(gsb, gs[:], cst[:, h:h + 1])
            nc.vector.tensor_scalar_add(gcb, gc[:], cct[:, h:h + 1])
            nc.vector.tensor_tensor(t1, gsb, cs[:, sl], op=MUL)
            nc.vector.tensor_tensor(t2, gcb, sn[:, sl], op=MUL)
            nc.vector.tensor_tensor(m2[:, sl], t1, t2, op=ADD)
            nc.vector.tensor_tensor(t1, gcb, cs[:, sl], op=MUL)
            nc.vector.tensor_tensor(t2, gsb, sn[:, sl], op=MUL)
            nc.vector.tensor_tensor(m3[:, sl], t1, t2, op=SUB)

        # ---- projections ----
        # Q and K psums are interleaved so their evictions drain on DIFFERENT
        # engines in parallel (Q -> DVE masked writes into the per-head
        # zero-padded qup tiles, K -> ACT copies into natural kTn tiles);
        # either alone outpaces PE and stalls the psum rotation.
        def emit_q_half(nchunk, icnk):
            p = ps_s.tile([128, 512], F32, tag="s")
            for kc in range(4):
                nc.tensor.matmul(
                    p[:],
                    wq[kc][:, nchunk * 128:(nchunk + 1) * 128],
                    xsT[kc][:, icnk * 512:(icnk + 1) * 512],
                    start=(kc == 0),
                    stop=(kc == 3),
                )
            for sub in range(2):
                h = 2 * nchunk + sub
                dst = qup[h][:, icnk * 512:(icnk + 1) * 512]
                # (psum * mask_h) + ubias_padded_h: writes the head's 64
                # real rows AND zeroes the sibling rows in one DVE op
                nc.vector.tensor_scalar(
                    dst, p[:], msk[:, h:h + 1], ubp[:, h:h + 1],
                    op0=MUL, op1=ADD)

        def emit_k_half(nchunk, jc):
            # K psums live in ps_g (idle during projections) so the Q
            # eviction backlog on ps_s never blocks the K matmuls
            p = ps_g.tile([128, 512], F32, tag="g")
            for kc in range(4):
                nc.tensor.matmul(
                    p[:],
                    wk[kc][:, nchunk * 128:(nchunk + 1) * 128],
                    xsT[kc][:, jc * 512:(jc + 1) * 512],
                    start=(kc == 0),
                    stop=(kc == 3),
                )
            nc.scalar.activation(
                kTn[nchunk][:, jc * 512:(jc + 1) * 512], p[:], Copy)

        # projection order: Q (ps_s psums, DVE masked evictions), then the
        # V-projection on the ps_z pool (idle until AV(0)) so PE keeps
        # streaming V matmuls while the Q eviction backlog drains, then K
        # (ps_s again, ACT evictions). G~(0)'s two chunks bracket V/K so
        # rope(0) rides DVE behind the Q evictions without blocking PE.
        for nchunk in range(4):
            emit_q_half(nchunk, 0)
            emit_q_half(nchunk, 1)

        m2_g0 = gpool.tile([128, T], DT, tag="m2", name="m2")
        m3_g0 = gpool.tile([128, T], DT, tag="m3", name="m3")
        tb_g0 = gpool.tile([128, 2048], DT, tag="ropet", name="ropet")
        gs_g0, gc_g0 = emit_g_mm(0, 0)
        emit_rope(0, 0, gs_g0, gc_g0, m2_g0, m3_g0, tb_g0)
        m2_cur, m3_cur = m2_g0, m3_g0

        # V[j,n] = sum_d xsT[d,j] Wv[d,n]; store with stride 65 into vp
        for jt in range(8):
            p = ps_z.tile([128, 512], F32, tag="z")
            for kc in range(4):
                nc.tensor.matmul(
                    p[:],
                    xsT[kc][:, jt * 128:(jt + 1) * 128],
                    wv[kc][:],
                    start=(kc == 0),
                    stop=(kc == 3),
                )
            dst = vp[:, jt * 520:(jt + 1) * 520].rearrange(
                "p (h x) -> p h x", h=8)[:, :, 0:64]
            src = p[:].rearrange("p (h x) -> p h x", h=8)
            nc.scalar.activation(dst, src, Copy)

        for nchunk in range(4):
            emit_k_half(nchunk, 0)
            emit_k_half(nchunk, 1)
            if nchunk == 1:
                gs_g1, gc_g1 = emit_g_mm(0, 1)
                emit_rope(0, 1, gs_g1, gc_g1, m2_g0, m3_g0, tb_g0)

        def emit_av_mm(h, icnk, jt, zp, attnT):
            nc.tensor.matmul(
                zp[:],
                vp[:, jt * 520 + 65 * h: jt * 520 + 65 * h + 65],
                attnT[:, jt * 1024 + icnk * 512:
                      jt * 1024 + icnk * 512 + 512],
                start=(jt == 0),
                stop=(jt == 7),
            )

        Ln = mybir.ActivationFunctionType.Ln

        def emit_znorm(h, icnk, zp):
            # 1/s computed as exp(-ln s) on ACT (DVE reciprocal is 8 cyc/elem)
            row = (h % 2) * 64
            lns = rpool.tile([1, 512], F32, tag="lns")
            nc.scalar.activation(lns[:], zp[64:65, :], Ln)
            rec = rpool.tile([1, 512], F32, tag="rec")
            nc.scalar.activation(rec[:], lns[:], Exp, scale=-1.0)
            recb = rpool.tile([64, 512], F32, tag="recb")
            nc.gpsimd.partition_broadcast(recb[:], rec[0:1, :])
            dst = zT[h // 2][row:row + 64, icnk * 512:(icnk + 1) * 512]
            nc.vector.tensor_tensor(dst, zp[0:64, :], recb[:], op=MUL)

        for h in range(H):
            attnT = apool.tile([128, 8192], DT, tag="attnT")
            zp0 = ps_z.tile([65, 512], F32, tag="z", name="zp0")
            zp1 = ps_z.tile([65, 512], F32, tag="z", name="zp1")
            if h + 1 < H:
                m2_nx = gpool.tile([128, T], DT, tag="m2", name="m2")
                m3_nx = gpool.tile([128, T], DT, tag="m3", name="m3")
                tb_nx = gpool.tile([128, 2048], DT, tag="ropet", name="ropet")
            # both i-chunks per jt: consecutive matmuls share each stationary
            # operand, halving the LDWEIGHTS issue pressure
            for jt in range(8):
                p0 = ps_s.tile([128, 512], F32, tag="s", name="p0")
                p1 = ps_s.tile([128, 512], F32, tag="s", name="p1")
                for icnk, p in ((0, p0), (1, p1)):
                    nc.tensor.matmul(
                        p[:],
                        kTn[h // 2][:, jt * 128:(jt + 1) * 128],
                        qup[h][:, icnk * 512:(icnk + 1) * 512],
                        start=True,
                        stop=False,
                    )
                for icnk, p in ((0, p0), (1, p1)):
                    nc.tensor.matmul(
                        p[:],
                        sn[:, jt * 128:(jt + 1) * 128],
                        m2_cur[:, icnk * 512:(icnk + 1) * 512],
                        start=False,
                        stop=False,
                    )
                for icnk, p in ((0, p0), (1, p1)):
                    nc.tensor.matmul(
                        p[:],
                        cs[:, jt * 128:(jt + 1) * 128],
                        m3_cur[:, icnk * 512:(icnk + 1) * 512],
                        start=False,
                        stop=True,
                    )
                for icnk, p in ((0, p0), (1, p1)):
                    nc.scalar.activation(
                        attnT[:, jt * 1024 + icnk * 512:
                              jt * 1024 + icnk * 512 + 512],
                        p[:], Exp, scale=float(SCALE))

                # pipeline next head's G~ matmuls + rope inside this head's
                # score loop (PE picks up the 2 small matmuls between score
                # tiles; rope rides on DVE)
                if h + 1 < H:
                    if jt == 0:
                        gs_nx0, gc_nx0 = emit_g_mm(h + 1, 0)
                    elif jt == 1:
                        emit_rope(h + 1, 0, gs_nx0, gc_nx0, m2_nx, m3_nx,
                                  tb_nx)
                    elif jt == 4:
                        gs_nx1, gc_nx1 = emit_g_mm(h + 1, 1)
                    elif jt == 5:
                        emit_rope(h + 1, 1, gs_nx1, gc_nx1, m2_nx, m3_nx,
                                  tb_nx)

            for jt in range(8):
                emit_av_mm(h, 0, jt, zp0, attnT)
            for jt in range(8):
                emit_av_mm(h, 1, jt, zp1, attnT)
            emit_znorm(h, 0, zp0)
            emit_znorm(h, 1, zp1)

            if h + 1 < H:
                m2_cur, m3_cur = m2_nx, m3_nx

        # ---- output projection ----
        # 4 psum tiles in flight per group; the zT[3] (heads 6/7) matmuls are
        # deferred to a second pass so the last head's znorm latency hides
        # behind the ncnk 0..2 accumulation matmuls
        for grp in range(2):
            ps = []
            for it4 in range(4):
                p = ps_s.tile([128, 512], F32, tag="s")
                ps.append(p)
                it = grp * 4 + it4
                for ncnk in range(3):
                    nc.tensor.matmul(
                        p[:],
                        zT[ncnk][:, it * 128:(it + 1) * 128],
                        wout[ncnk][:],
                        start=(ncnk == 0),
                        stop=False,
                    )
            for it4 in range(4):
                it = grp * 4 + it4
                p = ps[it4]
                nc.tensor.matmul(
                    p[:],
                    zT[3][:, it * 128:(it + 1) * 128],
                    wout[3][:],
                    start=False,
                    stop=True,
                )
                osb = opool.tile([128, 512], F32, tag="osb")
                nc.scalar.activation(osb[:], p[:], Copy)
                nc.sync.dma_start(out_d[it * 128:(it + 1) * 128, :], osb[:])

    nc.compile()
    _dedup_ldweights(nc)
    return nc


def _dedup_ldweights(nc):
    """Drop an InstLdweights when the immediately-preceding PE weight load
    (with only matmuls in between) loaded the identical stationary operand.
    Our paired score matmuls reuse each stationary operand twice; the
    duplicate load is what limits the PE instruction issue rate."""
    removed = 0
    for fn in nc.m.functions:
        for blk in fn.blocks:
            last_sig = None
            newlist = []
            for inst in blk.instructions:
                if isinstance(inst, mybir.InstLdweights):
                    sig = str(inst.ins[0])
                    si = inst.sync_info
                    clean = si is None or (
                        len(si.on_wait) == 0 and len(si.on_update) == 0)
                    if clean and sig == last_sig:
                        removed += 1
                        continue
                    last_sig = sig
                    newlist.append(inst)
                else:
                    newlist.append(inst)
            blk.instructions[:] = newlist
    return removed


def make_host_inputs(xs, Wq, Wk, Wv, Wpos, Wout, u_bias, v_bias, mode=MM_MODE):
    """Build the per-core input maps (host-side layout prep only)."""
    npdt = _np_dt(mode)
    nu, As, Ac = _freq_fit()

    ii = np.arange(T, dtype=np.float64)
    SN = np.sin(np.outer(nu, ii)).astype(np.float32)     # (M, T)
    CS = np.cos(np.outer(nu, ii)).astype(np.float32)

    # per-head folded weights: Wt_s_h = Wsin_h.T @ As.T  (64 x M), padded
    # into the 128-row head-pair coordinate (head h rows at (h%2)*64, rest 0)
    perm = np.concatenate([np.arange(0, D, 2), np.arange(1, D, 2)])
    Wpos_perm = np.asarray(Wpos, np.float64)[perm, :]     # (512, H*DH)
    WTS = np.zeros((128, H * M), np.float32)
    WTC = np.zeros((128, H * M), np.float32)
    csT = np.zeros((128, H), np.float32)
    ccT = np.zeros((128, H), np.float32)
    u64 = np.asarray(u_bias, np.float64)
    v64 = np.asarray(v_bias, np.float64)
    for h in range(H):
        Wsin = Wpos_perm[:256, h * DH:(h + 1) * DH]       # (256, 64)
        Wcos = Wpos_perm[256:, h * DH:(h + 1) * DH]
        Wt_s = Wsin.T @ As.T                              # (64, M)
        Wt_c = Wcos.T @ Ac.T
        row = (h % 2) * 64
        WTS[row:row + 64, h * M:(h + 1) * M] = Wt_s
        WTC[row:row + 64, h * M:(h + 1) * M] = Wt_c
        dvu = v64[h] - u64[h]                             # (64,)
        csT[:M, h] = dvu @ Wt_s
        ccT[:M, h] = dvu @ Wt_c

    # per-head zero-padded u_bias columns + row masks in head-pair coords
    ubp = np.zeros((128, H), np.float32)
    mskm = np.zeros((128, H), np.float32)
    for h in range(H):
        row = (h % 2) * 64
        ubp[row:row + 64, h] = np.asarray(u_bias, np.float32)[h]
        mskm[row:row + 64, h] = 1.0

    shared = {
        "Wq": np.ascontiguousarray(Wq).astype(npdt),
        "Wk": np.ascontiguousarray(Wk).astype(npdt),
        "Wv": np.ascontiguousarray(Wv).astype(npdt),
        "WTS": WTS.astype(npdt),
        "WTC": WTC.astype(npdt),
        "Wout": np.ascontiguousarray(Wout).astype(npdt),
        "ubp": ubp,
        "msk": mskm,
        "csT": csT,
        "ccT": ccT,
        "SN": SN.astype(npdt),
        "CS": CS.astype(npdt),
    }
    in_maps = []
    for b in range(B):
        m = dict(shared)
        m["xsT"] = np.ascontiguousarray(xs[b].T).astype(npdt)
        in_maps.append(m)
    return in_maps


_NC_CACHE = {}


def get_nc(mode=MM_MODE):
    if mode not in _NC_CACHE:
        _NC_CACHE[mode] = build_nc(mode)
    return _NC_CACHE[mode]


def _numpy_reference(xs, mask, Wq, Wk, Wv, Wpos, Wout, u_bias, v_bias):
    """Exact (fp32 numpy) fallback for non-all-ones masks."""
    b, t, _ = xs.shape
    pos = np.arange(-(t - 1), t, dtype=np.float32)[:, None]
    inv_freq = np.exp(-np.log(10000.0) *
                      np.arange(0, D, 2, dtype=np.float32) / D)
    angv = pos * inv_freq[None, :]
    pe = np.stack([np.sin(angv), np.cos(angv)], axis=-1).reshape(pos.shape[0], D)
    q = (xs @ Wq).reshape(b, t, H, DH).transpose(0, 2, 1, 3)
    k = (xs @ Wk).reshape(b, t, H, DH).transpose(0, 2, 1, 3)
    v = (xs @ Wv).reshape(b, t, H, DH).transpose(0, 2, 1, 3)
    p = (pe @ Wpos).reshape(-1, H, DH).transpose(1, 0, 2)
    q_u = q + u_bias[None, :, None, :]
    q_v = q + v_bias[None, :, None, :]
    ac = np.einsum("bhtd,bhsd->bhts", q_u, k)
    bd = np.einsum("bhtd,hld->bhtl", q_v, p)
    bdp = np.pad(bd, ((0, 0), (0, 0), (0, 0), (1, 0)))
    l = bd.shape[-1]
    bd = bdp.reshape(b, H, l + 1, t)[:, :, 1:, :].reshape(b, H, t, l)[..., :t]
    scores = (ac + bd) * SCALE
    m = (mask[:, None, :, :] == 0)
    scores = np.where(m, -np.inf, scores)
    scores = scores - scores.max(axis=-1, keepdims=True)
    e = np.exp(scores)
    attn = e / e.sum(axis=-1, keepdims=True)
    attn = np.where(m, 0.0, attn)
    z = np.einsum("bhts,bhsd->bthd", attn, v).reshape(b, t, H * DH)
    return (z @ Wout).astype(np.float32)


def kernel(xs, mask, Wq, Wk, Wv, Wpos, Wout, u_bias, v_bias):
    xs = np.asarray(xs, dtype=np.float32)
    mask = np.asarray(mask)
    Wq = np.asarray(Wq, dtype=np.float32)
    Wk = np.asarray(Wk, dtype=np.float32)
    Wv = np.asarray(Wv, dtype=np.float32)
    Wpos = np.asarray(Wpos, dtype=np.float32)
    Wout = np.asarray(Wout, dtype=np.float32)
    u_bias = np.asarray(u_bias, dtype=np.float32)
    v_bias = np.asarray(v_bias, dtype=np.float32)

    if not np.all(mask != 0):
        # the on-device kernel assumes the (spec-pinned) all-ones mask
        return _numpy_reference(xs, mask, Wq, Wk, Wv, Wpos, Wout, u_bias, v_bias)

    nc = get_nc(MM_MODE)
    in_maps = make_host_inputs(xs, Wq, Wk, Wv, Wpos, Wout, u_bias, v_bias,
                               MM_MODE)
    res = run_bass_kernel_spmd(nc, in_maps, core_ids=list(range(NCORES)))
    out = np.stack([np.asarray(res.results[b]["out"], dtype=np.float32)
                    for b in range(B)], axis=0)
    return out


if __name__ == "__main__":
    # smoke-test: build only
    nc = build_nc()
    print("build ok")
